# revision 18
# baseline (speedup 1.0000x reference)
"""Trainium2 Bass kernel for linear attention over external memory.

Computes out = x @ (keys^T @ vals) for
  x [4, 2048, 1024] f32, keys/vals [65536, 1024] f32.

Sharding across 8 NeuronCores: keys/vals sharded along the memory dim M
(8192 rows per core); x sharded by token (1024 rows per core).

v5 design:
  - Host pre-casts keys/vals to bf16 (halves the HBM stream, enables
    the PE fast-weight-load path) and pre-transposes x per core to
    xT [D, T] bf16, so the kernel needs no PE transposes.
  - Stage 2 runs as four streaming passes over the memory chunks,
    each accumulating one column half of kv directly in PSUM:
    A-h0 (chunks 0..39), A-h1 (same chunks, keys re-read), B-h0
    (chunks 40..63), B-h1. Each pass ends with ACT casts PSUM->bf16.
  - The kv reduction is split into two partial AllReduces: AR_A (2MB)
    after the A passes hides under the B passes; AR_B0 (1MB) hides
    under the B-h1 pass; only AR_B1 is partially exposed, overlapped
    with stage 4 on the h0 half.
  - Stage 4 (out = x @ kv) consumes each column half as soon as its
    two partial results are readback-summed.
"""

import numpy as np

# Problem shapes (hardcoded per contract).
B, S, D = 4, 2048, 1024
M = 65536
NCORES = 8
P = 128
T = (B * S) // NCORES          # 1024 tokens per core
KM = M // NCORES               # 8192 memory rows per core
NC_ = KM // P                  # 64 k-chunks
NCA = 32                       # phase-A chunks
NCB = NC_ - NCA                # 24 phase-B chunks
DB = D // P                    # 8 d-blocks
HALF = D // 2                  # 512
TCH = T // P                   # 8 token chunks
JTAIL = 8                      # per-pass tail chunks ordered j-outer

_CACHE = {}


def _build_nc():
    import concourse.bacc as bacc
    import concourse.tile as tile
    from concourse import mybir

    f32 = mybir.dt.float32
    bf16 = mybir.dt.bfloat16
    ACT_COPY = mybir.ActivationFunctionType.Copy

    nc = bacc.Bacc("TRN2", target_bir_lowering=False, debug=False,
                   num_devices=NCORES)

    KVW = D + HALF                                        # 1536
    xt_d = nc.dram_tensor("xt", [D, T], bf16, kind="ExternalInput")
    kv0_d = nc.dram_tensor("kvh0", [KM, KVW], bf16, kind="ExternalInput")
    kv1_d = nc.dram_tensor("kvh1", [KM, KVW], bf16, kind="ExternalInput")
    out_d = nc.dram_tensor("out", [T, D], f32, kind="ExternalOutput")

    kv_r = [kv0_d.ap().rearrange("(c p) n -> c p n", p=P),  # [64,128,1536]
            kv1_d.ap().rearrange("(c p) n -> c p n", p=P)]
    xt_r = xt_d.ap().rearrange("(j p) t -> j p t", p=P)   # [8, 128, 1024]

    with tile.TileContext(nc) as tc:
        with (
            tc.tile_pool(name="const", bufs=1) as const,
            tc.tile_pool(name="kvp", bufs=32) as kvp,
            tc.tile_pool(name="xtp", bufs=DB) as xtp,
            tc.tile_pool(name="kvev", bufs=2) as kvev,
            tc.tile_pool(name="kvfin", bufs=2) as kvfin,
            tc.tile_pool(name="outp", bufs=3) as outp,
            tc.tile_pool(name="ps", bufs=8, space="PSUM") as ps,
            tc.tile_pool(name="dram", bufs=8, space="DRAM") as dram,
        ):
            # Warm-up collective: arms the ncfw collective stream so the
            # first real AllReduce trigger doesn't pay the wake-up.
            warm = const.tile([P, 16], bf16)
            nc.gpsimd.memset(warm[:], 0.0)
            warm_in = dram.tile([P, 16], bf16, name="warm_in")
            warm_out = dram.tile([P, 16], bf16, name="warm_out",
                                 addr_space="Shared")
            nc.gpsimd.dma_start(out=warm_in[:], in_=warm[:])
            nc.gpsimd.collective_compute(
                "AllReduce",
                mybir.AluOpType.add,
                replica_groups=[list(range(NCORES))],
                ins=[warm_in.opt()],
                outs=[warm_out.opt()],
            )

            xT = [xtp.tile([P, T], bf16, name=f"xT{j}", tag="xT")
                  for j in range(DB)]

            # ---- stage 2: four streaming passes, PSUM-accumulated ----
            # Each pass covers one column half over a chunk range; the
            # cast of each PSUM tile to its bf16 AllReduce staging slice
            # pipelines via a j-outer ordering of the last chunks.
            def stage2_pass(c0, ncnt, h, kvev_tile):
                pst = [ps.tile([P, HALF], f32, name=f"p{h}_{j}",
                               tag="ps") for j in range(DB)]
                ntail = min(JTAIL, ncnt)
                nhead = ncnt - ntail
                tiles = []
                for c in range(ncnt):
                    kvt = kvp.tile([P, KVW], bf16, name="kvt", tag="kvt")
                    nc.sync.dma_start(out=kvt[:], in_=kv_r[h][c0 + c])
                    tiles.append(kvt)
                    if c < nhead:
                        for j in range(DB):
                            nc.tensor.matmul(
                                pst[j][:],
                                kvt[:, j * P:(j + 1) * P],
                                kvt[:, D:D + HALF],
                                start=(c == 0), stop=False)
                # tail chunks j-outer so each PSUM tile finishes early
                for j in range(DB):
                    for c in range(nhead, ncnt):
                        kvt = tiles[c]
                        nc.tensor.matmul(
                            pst[j][:],
                            kvt[:, j * P:(j + 1) * P],
                            kvt[:, D:D + HALF],
                            start=(c == 0 and nhead == 0),
                            stop=(c == ncnt - 1))
                    sl = slice(j * HALF, (j + 1) * HALF)
                    nc.scalar.activation(kvev_tile[:, sl], pst[j][:],
                                         ACT_COPY)

            def bounce_and_ar(kvev_tile, bin_t, bout_t, off):
                for j in range(DB):
                    sl = slice(off + j * HALF, off + (j + 1) * HALF)
                    nc.gpsimd.dma_start(out=bin_t[:, sl],
                                        in_=kvev_tile[:, sl.start - off:
                                                      sl.stop - off])

            # Phase A: both halves, then one 2MB AllReduce.
            kvevA = [kvev.tile([P, DB * HALF], bf16, name=f"kvevA{h}",
                               tag="kvev") for h in range(2)]
            binA = dram.tile([P, 2 * DB * HALF], bf16, name="binA")
            boutA = dram.tile([P, 2 * DB * HALF], bf16, name="boutA",
                              addr_space="Shared")
            for h in range(2):
                stage2_pass(0, NCA, h, kvevA[h])
                if h == 0:
                    # xT loads slot in after the first pass's loads.
                    for j in range(DB):
                        nc.sync.dma_start(out=xT[j][:], in_=xt_r[j])
                bounce_and_ar(kvevA[h], binA, boutA, h * DB * HALF)
            nc.gpsimd.collective_compute(
                "AllReduce",
                mybir.AluOpType.add,
                replica_groups=[list(range(NCORES))],
                ins=[binA.opt()],
                outs=[boutA.opt()],
            )

            # Phase B: per-half pass + 1MB AllReduce each.
            binB = [dram.tile([P, DB * HALF], bf16, name=f"binB{h}")
                    for h in range(2)]
            boutB = [dram.tile([P, DB * HALF], bf16, name=f"boutB{h}",
                               addr_space="Shared") for h in range(2)]
            for h in range(2):
                kvevBh = kvev.tile([P, DB * HALF], bf16, name=f"kvevB{h}",
                                   tag="kvev")
                stage2_pass(NCA, NCB, h, kvevBh)
                bounce_and_ar(kvevBh, binB[h], boutB[h], 0)
                nc.gpsimd.collective_compute(
                    "AllReduce",
                    mybir.AluOpType.add,
                    replica_groups=[list(range(NCORES))],
                    ins=[binB[h].opt()],
                    outs=[boutB[h].opt()],
                )

            # Readback of the A result (sync queue, after all loads).
            kvf = [kvfin.tile([P, DB * HALF], bf16, name=f"kvf{h}",
                              tag="kvfin") for h in range(2)]
            for h in range(2):
                nc.sync.dma_start(
                    out=kvf[h][:],
                    in_=boutA[:, h * DB * HALF:(h + 1) * DB * HALF])

            # ---- stage 4: out = x @ (kvA + kvB), per column half ----
            for h in range(2):
                kvBr = kvev.tile([P, DB * HALF], bf16, name=f"kvBr{h}",
                                 tag="kvev")
                # Per-slice readback+add so stage 4's j-accumulation can
                # start ~1us after the AllReduce lands.
                for j in range(DB):
                    sl = slice(j * HALF, (j + 1) * HALF)
                    nc.sync.dma_start(out=kvBr[:, sl],
                                      in_=boutB[h][:, sl])
                    nc.vector.tensor_tensor(
                        out=kvf[h][:, sl], in0=kvBr[:, sl],
                        in1=kvf[h][:, sl],
                        op=mybir.AluOpType.add)
                for i in range(TCH):
                    po = ps.tile([P, HALF], f32, name="po", tag="ps")
                    for j in range(DB):
                        nc.tensor.matmul(
                            po[:],
                            xT[j][:, i * P:(i + 1) * P],
                            kvf[h][:, j * HALF:(j + 1) * HALF],
                            start=(j == 0), stop=(j == DB - 1))
                    ob = outp.tile([P, HALF], f32, name="ob", tag="ob")
                    nc.scalar.activation(ob[:], po[:], ACT_COPY)
                    nc.sync.dma_start(
                        out=out_d.ap()[i * P:(i + 1) * P,
                                       h * HALF:(h + 1) * HALF],
                        in_=ob[:])

    nc.compile()
    return nc


def _get_nc():
    if "nc" not in _CACHE:
        _CACHE["nc"] = _build_nc()
    return _CACHE["nc"]


def kernel(**inputs):
    import ml_dtypes
    from concourse.bass_utils import run_bass_kernel_spmd

    bf16 = ml_dtypes.bfloat16
    x = np.asarray(inputs["x"], dtype=np.float32)
    keys = np.asarray(inputs["keys"], dtype=np.float32)
    vals = np.asarray(inputs["vals"], dtype=np.float32)
    xf = x.reshape(B * S, D)

    keys_b = keys.astype(bf16)
    vals_b = vals.astype(bf16)

    nc = _get_nc()
    in_maps = []
    for c in range(NCORES):
        xt = np.ascontiguousarray(
            xf[c * T:(c + 1) * T].T.astype(bf16))       # [D, T] bf16
        kb = keys_b[c * KM:(c + 1) * KM]
        vb = vals_b[c * KM:(c + 1) * KM]
        in_maps.append({
            "xt": xt,
            # per-half packed [k | v-half] rows: one 3KB-line DMA/chunk
            "kvh0": np.ascontiguousarray(
                np.concatenate([kb, vb[:, :HALF]], axis=1)),
            "kvh1": np.ascontiguousarray(
                np.concatenate([kb, vb[:, HALF:]], axis=1)),
        })
    res = run_bass_kernel_spmd(nc, in_maps, list(range(NCORES)))
    out = np.concatenate([res.results[c]["out"] for c in range(NCORES)],
                         axis=0)
    return out.reshape(B, S, D).astype(np.float32)


# revision 19
# speedup vs baseline: 1.0123x; 1.0123x over previous
"""Trainium2 Bass kernel for linear attention over external memory.

Computes out = x @ (keys^T @ vals) for
  x [4, 2048, 1024] f32, keys/vals [65536, 1024] f32.

Sharding across 8 NeuronCores: keys/vals sharded along the memory dim M
(8192 rows per core); x sharded by token (1024 rows per core).

v5 design:
  - Host pre-casts keys/vals to bf16 (halves the HBM stream, enables
    the PE fast-weight-load path) and pre-transposes x per core to
    xT [D, T] bf16, so the kernel needs no PE transposes.
  - Stage 2 runs as four streaming passes over the memory chunks,
    each accumulating one column half of kv directly in PSUM:
    A-h0 (chunks 0..39), A-h1 (same chunks, keys re-read), B-h0
    (chunks 40..63), B-h1. Each pass ends with ACT casts PSUM->bf16.
  - The kv reduction is split into two partial AllReduces: AR_A (2MB)
    after the A passes hides under the B passes; AR_B0 (1MB) hides
    under the B-h1 pass; only AR_B1 is partially exposed, overlapped
    with stage 4 on the h0 half.
  - Stage 4 (out = x @ kv) consumes each column half as soon as its
    two partial results are readback-summed.
"""

import numpy as np

# Problem shapes (hardcoded per contract).
B, S, D = 4, 2048, 1024
M = 65536
NCORES = 8
P = 128
T = (B * S) // NCORES          # 1024 tokens per core
KM = M // NCORES               # 8192 memory rows per core
NC_ = KM // P                  # 64 k-chunks
NCA = 32                       # phase-A chunks
NCB = NC_ - NCA                # 24 phase-B chunks
DB = D // P                    # 8 d-blocks
HALF = D // 2                  # 512
TCH = T // P                   # 8 token chunks
JTAIL = 8                      # per-pass tail chunks ordered j-outer

_CACHE = {}


def _build_nc():
    import concourse.bacc as bacc
    import concourse.tile as tile
    from concourse import mybir

    f32 = mybir.dt.float32
    bf16 = mybir.dt.bfloat16
    ACT_COPY = mybir.ActivationFunctionType.Copy

    nc = bacc.Bacc("TRN2", target_bir_lowering=False, debug=False,
                   num_devices=NCORES)

    KVW = D + HALF                                        # 1536
    xt_d = nc.dram_tensor("xt", [D, T], bf16, kind="ExternalInput")
    kv0_d = nc.dram_tensor("kvh0", [KM, KVW], bf16, kind="ExternalInput")
    kv1_d = nc.dram_tensor("kvh1", [KM, KVW], bf16, kind="ExternalInput")
    out_d = nc.dram_tensor("out", [T, D], f32, kind="ExternalOutput")

    kv_r = [kv0_d.ap().rearrange("(c p) n -> c p n", p=P),  # [64,128,1536]
            kv1_d.ap().rearrange("(c p) n -> c p n", p=P)]
    xt_r = xt_d.ap().rearrange("(j p) t -> j p t", p=P)   # [8, 128, 1024]

    with tile.TileContext(nc) as tc:
        with (
            tc.tile_pool(name="const", bufs=1) as const,
            tc.tile_pool(name="kvp", bufs=32) as kvp,
            tc.tile_pool(name="xtp", bufs=DB) as xtp,
            tc.tile_pool(name="kvev", bufs=2) as kvev,
            tc.tile_pool(name="kvfin", bufs=2) as kvfin,
            tc.tile_pool(name="outp", bufs=3) as outp,
            tc.tile_pool(name="ps", bufs=8, space="PSUM") as ps,
            tc.tile_pool(name="dram", bufs=8, space="DRAM") as dram,
        ):
            # Warm-up collective: arms the ncfw collective stream so the
            # first real AllReduce trigger doesn't pay the wake-up.
            warm = const.tile([P, 16], bf16)
            nc.gpsimd.memset(warm[:], 0.0)
            warm_in = dram.tile([P, 16], bf16, name="warm_in")
            warm_out = dram.tile([P, 16], bf16, name="warm_out",
                                 addr_space="Shared")
            nc.gpsimd.dma_start(out=warm_in[:], in_=warm[:])
            nc.gpsimd.collective_compute(
                "AllReduce",
                mybir.AluOpType.add,
                replica_groups=[list(range(NCORES))],
                ins=[warm_in.opt()],
                outs=[warm_out.opt()],
            )

            xT = [xtp.tile([P, T], bf16, name=f"xT{j}", tag="xT")
                  for j in range(DB)]

            # ---- stage 2: four streaming passes, PSUM-accumulated ----
            # Each pass covers one column half over a chunk range; the
            # cast of each PSUM tile to its bf16 AllReduce staging slice
            # pipelines via a j-outer ordering of the last chunks.
            def stage2_pass(c0, ncnt, h, kvev_tile):
                pst = [ps.tile([P, HALF], f32, name=f"p{h}_{j}",
                               tag="ps") for j in range(DB)]
                ntail = min(JTAIL, ncnt)
                nhead = ncnt - ntail
                tiles = []
                for c in range(ncnt):
                    kvt = kvp.tile([P, KVW], bf16, name="kvt", tag="kvt")
                    nc.sync.dma_start(out=kvt[:], in_=kv_r[h][c0 + c])
                    tiles.append(kvt)
                    if c < nhead:
                        for j in range(DB):
                            nc.tensor.matmul(
                                pst[j][:],
                                kvt[:, j * P:(j + 1) * P],
                                kvt[:, D:D + HALF],
                                start=(c == 0), stop=False)
                # tail chunks j-outer so each PSUM tile finishes early
                for j in range(DB):
                    for c in range(nhead, ncnt):
                        kvt = tiles[c]
                        nc.tensor.matmul(
                            pst[j][:],
                            kvt[:, j * P:(j + 1) * P],
                            kvt[:, D:D + HALF],
                            start=(c == 0 and nhead == 0),
                            stop=(c == ncnt - 1))
                    sl = slice(j * HALF, (j + 1) * HALF)
                    nc.scalar.activation(kvev_tile[:, sl], pst[j][:],
                                         ACT_COPY)

            def bounce_and_ar(kvev_tile, bin_t, bout_t, off):
                for j in range(DB):
                    sl = slice(off + j * HALF, off + (j + 1) * HALF)
                    nc.gpsimd.dma_start(out=bin_t[:, sl],
                                        in_=kvev_tile[:, sl.start - off:
                                                      sl.stop - off])

            # Phase A: both halves, then one 2MB AllReduce.
            kvevA = [kvev.tile([P, DB * HALF], bf16, name=f"kvevA{h}",
                               tag="kvev") for h in range(2)]
            binA = dram.tile([P, 2 * DB * HALF], bf16, name="binA")
            boutA = dram.tile([P, 2 * DB * HALF], bf16, name="boutA",
                              addr_space="Shared")
            for h in range(2):
                stage2_pass(0, NCA, h, kvevA[h])
                if h == 0:
                    # xT loads slot in after the first pass's loads.
                    for j in range(DB):
                        nc.sync.dma_start(out=xT[j][:], in_=xt_r[j])
                bounce_and_ar(kvevA[h], binA, boutA, h * DB * HALF)
            nc.gpsimd.collective_compute(
                "AllReduce",
                mybir.AluOpType.add,
                replica_groups=[list(range(NCORES))],
                ins=[binA.opt()],
                outs=[boutA.opt()],
            )

            # Phase B: per-half pass + 1MB AllReduce each.
            binB = [dram.tile([P, DB * HALF], bf16, name=f"binB{h}")
                    for h in range(2)]
            boutB = [dram.tile([P, DB * HALF], bf16, name=f"boutB{h}",
                               addr_space="Shared") for h in range(2)]
            for h in range(2):
                kvevBh = kvev.tile([P, DB * HALF], bf16, name=f"kvevB{h}",
                                   tag="kvev")
                stage2_pass(NCA, NCB, h, kvevBh)
                bounce_and_ar(kvevBh, binB[h], boutB[h], 0)
                nc.gpsimd.collective_compute(
                    "AllReduce",
                    mybir.AluOpType.add,
                    replica_groups=[list(range(NCORES))],
                    ins=[binB[h].opt()],
                    outs=[boutB[h].opt()],
                )

            # Readback of the A result (sync queue, after all loads).
            kvf = [kvfin.tile([P, DB * HALF], bf16, name=f"kvf{h}",
                              tag="kvfin") for h in range(2)]
            for h in range(2):
                nc.sync.dma_start(
                    out=kvf[h][:],
                    in_=boutA[:, h * DB * HALF:(h + 1) * DB * HALF])

            # ---- stage 4: out = x @ (kvA + kvB), per column half ----
            for h in range(2):
                kvBr = kvev.tile([P, DB * HALF], bf16, name=f"kvBr{h}",
                                 tag="kvev")
                nc.sync.dma_start(out=kvBr[:], in_=boutB[h][:])
                nc.vector.tensor_tensor(
                    out=kvf[h][:], in0=kvBr[:], in1=kvf[h][:],
                    op=mybir.AluOpType.add)
                for i in range(TCH):
                    po = ps.tile([P, HALF], f32, name="po", tag="ps")
                    for j in range(DB):
                        nc.tensor.matmul(
                            po[:],
                            xT[j][:, i * P:(i + 1) * P],
                            kvf[h][:, j * HALF:(j + 1) * HALF],
                            start=(j == 0), stop=(j == DB - 1))
                    ob = outp.tile([P, HALF], f32, name="ob", tag="ob")
                    nc.scalar.activation(ob[:], po[:], ACT_COPY)
                    nc.sync.dma_start(
                        out=out_d.ap()[i * P:(i + 1) * P,
                                       h * HALF:(h + 1) * HALF],
                        in_=ob[:])

    nc.compile()
    return nc


def _get_nc():
    if "nc" not in _CACHE:
        _CACHE["nc"] = _build_nc()
    return _CACHE["nc"]


def kernel(**inputs):
    import ml_dtypes
    from concourse.bass_utils import run_bass_kernel_spmd

    bf16 = ml_dtypes.bfloat16
    x = np.asarray(inputs["x"], dtype=np.float32)
    keys = np.asarray(inputs["keys"], dtype=np.float32)
    vals = np.asarray(inputs["vals"], dtype=np.float32)
    xf = x.reshape(B * S, D)

    keys_b = keys.astype(bf16)
    vals_b = vals.astype(bf16)

    nc = _get_nc()
    in_maps = []
    for c in range(NCORES):
        xt = np.ascontiguousarray(
            xf[c * T:(c + 1) * T].T.astype(bf16))       # [D, T] bf16
        kb = keys_b[c * KM:(c + 1) * KM]
        vb = vals_b[c * KM:(c + 1) * KM]
        in_maps.append({
            "xt": xt,
            # per-half packed [k | v-half] rows: one 3KB-line DMA/chunk
            "kvh0": np.ascontiguousarray(
                np.concatenate([kb, vb[:, :HALF]], axis=1)),
            "kvh1": np.ascontiguousarray(
                np.concatenate([kb, vb[:, HALF:]], axis=1)),
        })
    res = run_bass_kernel_spmd(nc, in_maps, list(range(NCORES)))
    out = np.concatenate([res.results[c]["out"] for c in range(NCORES)],
                         axis=0)
    return out.reshape(B, S, D).astype(np.float32)
